# revision 1
# baseline (speedup 1.0000x reference)
"""Distributed GATv2 message-passing kernel for 8 Trainium2 NeuronCores.

Sharding: nodes (and their incoming edges) are partitioned across the 8
cores by dst-node chunk of 1024; GATv2 weights are replicated; node
features are exchanged once per layer with an fp8 AllGather.

Per core / per layer on device:
  - one dma_gather per 128-dst-node group fetches transposed fp8 x rows
    for the group's (padded) edge sources AND the group's own 128 nodes;
    the gather's 16-bit-granularity transpose leaves fp8 k-index PAIRS
    per partition, which is exactly the DoubleRow matmul operand layout
  - xr = x_own @ Wr and m = X_src@Wl (+ one-hot rel_proj/xr adds) run as
    fp8e4 DoubleRow matmuls (2 k-tiles per instruction, 2x PE rate);
    weights are host-scaled by 64 to avoid fp8 denormals and unscaled
    via the ACT activation scale / the softmax reciprocal fold
  - logits = att . leaky_relu(m) via ACT Lrelu chunks into one big tile,
    one big multiply by att, one 3D tensor_reduce per edge tile
  - segment softmax + weighted aggregation via one-hot matmuls (the
    edge->node-tile assignment matrices are shipped as static data)
  - pre-gelu activations are staged; one deferred pass per layer applies
    gelu + residual (avoids ACT table-set thrash between Exp and Gelu)
Final: gate/fuse with x_text, Wp projection, layernorm, gelu.

Edges are sorted by dst and packed into 128-wide tiles grouped by
128-node dst tile; tiles-per-group is padded to a global constant so one
SPMD program serves all 8 cores (per-core differences live in the data:
gather indices and one-hot matrices).
"""
import sys
sys.path.insert(0, "/opt/trn_rl_repo")

import numpy as np
import ml_dtypes

import concourse.bass as bass
import concourse.bacc as bacc
import concourse.mybir as mybir
import concourse.tile as tile
from concourse import library_config
from concourse.bass_utils import run_bass_kernel_spmd

AF = mybir.ActivationFunctionType
OP = mybir.AluOpType
dt = mybir.dt
AX = mybir.AxisListType
PM = mybir.MatmulPerfMode

N, D, E, L, H, R = 8192, 768, 16384, 3, 4, 64
NC = 8            # cores
CHN = N // NC     # 1024 nodes per core
NT = CHN // 128   # 8 node tiles per core
HD = H * D        # 3072
KT = D // 128     # 6 contraction tiles (bf16)
KT2 = KT // 2     # 3 fp8 DoubleRow contraction pairs
NCH = 6           # column chunks of 512
CW = HD // NCH    # 512
EPS_LN = 1e-5
DEN_EPS = 1e-6
WSC = 64.0        # fp8 weight scale (avoids e4m3 denormals)

bf16 = ml_dtypes.bfloat16
f8 = ml_dtypes.float8_e4m3


# ---------------------------------------------------------------- host prep
def _preprocess(x_text, rel_emb, Wl, bl, Wr, br, We, att, bout,
                Wg, bg, Wp, bp, gamma, beta, edge_index, edge_attr):
    src_all = np.asarray(edge_index[0], np.int64)
    dst_all = np.asarray(edge_index[1], np.int64)
    rel_all = np.asarray(edge_attr, np.int64)

    # node row layout of the exchange buffer (and x_full): halves-major
    # [half, core, 512, :] so each half-AllGather writes contiguously
    nn = np.arange(N)
    rmap = ((nn % CHN) >= CHN // 2) * (N // 2) + \
        (nn // CHN) * (CHN // 2) + (nn % (CHN // 2))

    per_core = []
    max_tiles = 1
    for c in range(NC):
        sel = np.nonzero((dst_all >= c * CHN) & (dst_all < (c + 1) * CHN))[0]
        order = np.argsort(dst_all[sel], kind="stable")
        sel = sel[order]
        dloc = dst_all[sel] - c * CHN
        groups = []
        for g in range(NT):
            gsel = sel[(dloc >= g * 128) & (dloc < (g + 1) * 128)]
            groups.append(gsel)
            max_tiles = max(max_tiles, (len(gsel) + 127) // 128)
        per_core.append(groups)
    TG = max_tiles               # edge tiles per node-tile group (uniform)
    ET = NT * TG                 # edge tiles per core
    GE = TG * 128                # padded edges per group
    GI = GE + 128                # gather idxs per group (edges + own nodes)
    EPI = NT * GI                # gather idxs per core

    in_maps = []
    w_shared = None
    for c in range(NC):
        gidx = np.zeros(EPI, np.int16)
        oneD = np.zeros((128, ET, 128), bf16)   # [dst_local, tile, e]
        oneA = np.zeros((128, ET, 128), bf16)   # [e, tile, dst_local]
        oneR = np.zeros((R, ET, 128), bf16)     # [rel, tile, e]
        for g in range(NT):
            ge = per_core[c][g]
            for i, eidx in enumerate(ge):
                t = g * TG + i // 128
                e = i % 128
                nl = int(dst_all[eidx]) - c * CHN - g * 128
                # each 128-idx block is stored REVERSED: the SwInterleave
                # matmul mode reads stationary columns in reverse order,
                # so out partition e lands on original edge e
                gidx[g * GI + (i // 128) * 128 + (127 - e)] = rmap[src_all[eidx]]
                oneD[nl, t, e] = 1
                oneA[e, t, nl] = 1
                oneR[int(rel_all[eidx]), t, e] = 1
            # own 128 node rows appended after the group's edge tiles
            own0 = c * CHN + g * 128
            gidx[g * GI + GE:g * GI + GI] = rmap[
                np.arange(own0 + 127, own0 - 1, -1)].astype(np.int16)
        idx_w = np.tile(gidx.reshape(EPI // 16, 16).T, (8, 1)).copy()

        if w_shared is None:
            # weight tensors, cast/reshaped only (shared by all cores)
            def ktile(w):  # [768, X] -> [128, 6, X]
                return np.ascontiguousarray(
                    w.reshape(KT, 128, -1).transpose(1, 0, 2)).astype(bf16)

            def k8tile(w):  # [768, X] -> [128, 3, 2, X] fp8, x64 scale
                w = np.asarray(w, np.float32) * WSC
                w = w.reshape(KT2, 128, 2, -1)   # k = 256c + 2p + j
                return np.ascontiguousarray(
                    w.transpose(1, 0, 2, 3)).astype(f8)

            wl8 = np.stack([k8tile(np.asarray(Wl[l])) for l in range(L)])
            wr8 = np.stack([k8tile(np.asarray(Wr[l])) for l in range(L)])
            # We and rel_emb both x64 (fp8): relp psum lands at 4096x and
            # is rescaled to 64x units during the bias add
            we8 = np.stack([k8tile(np.asarray(We[l])) for l in range(L)])
            relT8 = k8tile(np.asarray(rel_emb).T)        # [128, 3, 2, 64]
            wg1_rep = np.ascontiguousarray(np.broadcast_to(
                np.asarray(Wg)[0:D, 0][None, :], (128, D))).astype(bf16)
            wg2_rep = np.ascontiguousarray(np.broadcast_to(
                np.asarray(Wg)[D:2 * D, 0][None, :], (128, D))).astype(bf16)
            ident = np.eye(128, dtype=bf16)
            wp_t = ktile(np.asarray(Wp))                 # [128, 6, 768]
            attm = np.asarray(att).reshape(L, HD)        # [L, 3072]
            att_rep = np.broadcast_to(
                WSC * attm[:, None, :], (L, 128, HD)).astype(bf16)
            blbr_rep = np.broadcast_to(
                WSC * (np.asarray(bl) + np.asarray(br))[:, None, :],
                (L, 128, HD)).astype(bf16)
            # blm = 0.25 * sum_h bl[h]  (isolated-node xl bias fallback)
            blm = 0.25 * np.asarray(bl).reshape(L, H, D).sum(1)
            blm_rep = np.broadcast_to(
                blm[:, None, :], (L, 128, D)).astype(np.float32)
            bout_rep = np.broadcast_to(
                np.asarray(bout)[:, None, :], (L, 128, D)).astype(np.float32)
            bg_rep = np.full((128, 1), float(np.asarray(bg)[0]), np.float32)
            bp_rep = np.broadcast_to(
                np.asarray(bp)[None, :], (128, D)).astype(np.float32)
            gamma_rep = np.broadcast_to(
                np.asarray(gamma)[None, :], (128, D)).astype(np.float32)
            beta_rep = np.broadcast_to(
                np.asarray(beta)[None, :], (128, D)).astype(np.float32)
            xf32 = np.asarray(x_text, np.float32)
            x8 = xf32.astype(f8)
            x8r = np.empty_like(x8)
            x8r[rmap] = x8
            x_full8 = np.ascontiguousarray(x8r).view(bf16)
            w_shared = dict(
                wl8=np.ascontiguousarray(wl8), wr8=np.ascontiguousarray(wr8),
                we8=np.ascontiguousarray(we8), relT8=np.ascontiguousarray(relT8),
                wg1_rep=wg1_rep, wg2_rep=wg2_rep, ident=ident,
                wp=np.ascontiguousarray(wp_t),
                att_rep=np.ascontiguousarray(att_rep),
                blbr_rep=np.ascontiguousarray(blbr_rep),
                blm_rep=np.ascontiguousarray(blm_rep),
                bout_rep=np.ascontiguousarray(bout_rep),
                bg_rep=bg_rep, bp_rep=np.ascontiguousarray(bp_rep),
                gamma_rep=np.ascontiguousarray(gamma_rep),
                beta_rep=np.ascontiguousarray(beta_rep),
                x_full=np.ascontiguousarray(x_full8),
            )
        x_own = np.ascontiguousarray(np.asarray(
            x_text[c * CHN:(c + 1) * CHN], np.float32))
        m = dict(w_shared)
        m.update(x_own=x_own,
                 src_idx=idx_w, oneD=oneD, oneA=oneA, oneR=oneR)
        in_maps.append(m)
    return in_maps, TG


# ---------------------------------------------------------------- device
def build_program(TG, repeat=1, abl=frozenset()):
    ET = NT * TG
    GE = TG * 128
    GI = GE + 128
    EPI = NT * GI
    D2 = D // 2
    nc = bacc.Bacc("TRN2", target_bir_lowering=False, debug=False,
                   num_devices=NC)

    def inp(name, shape, dtype):
        return nc.dram_tensor(name, list(shape), dtype, kind="ExternalInput")

    x_full = inp("x_full", [N, D2], dt.bfloat16)   # fp8 bytes, bf16-typed
    x_own = inp("x_own", [CHN, D], dt.float32)
    wl8_d = inp("wl8", [L, 128, KT2, 2, HD], dt.float8e4)
    wr8_d = inp("wr8", [L, 128, KT2, 2, HD], dt.float8e4)
    we_d = inp("we8", [L, 128, KT2, 2, HD], dt.float8e4)
    relT_d = inp("relT8", [128, KT2, 2, R], dt.float8e4)
    wg1_d = inp("wg1_rep", [128, D], dt.bfloat16)
    wg2_d = inp("wg2_rep", [128, D], dt.bfloat16)
    ident_d = inp("ident", [128, 128], dt.bfloat16)
    wp_d = inp("wp", [128, KT, D], dt.bfloat16)
    att_d = inp("att_rep", [L, 128, HD], dt.bfloat16)
    blbr_d = inp("blbr_rep", [L, 128, HD], dt.bfloat16)
    blm_d = inp("blm_rep", [L, 128, D], dt.float32)
    bor_d = inp("bout_rep", [L, 128, D], dt.float32)
    bgr_d = inp("bg_rep", [128, 1], dt.float32)
    bpr_d = inp("bp_rep", [128, D], dt.float32)
    gmr_d = inp("gamma_rep", [128, D], dt.float32)
    btr_d = inp("beta_rep", [128, D], dt.float32)
    idx_d = inp("src_idx", [128, EPI // 16], dt.int16)
    oneD_d = inp("oneD", [128, ET, 128], dt.bfloat16)
    oneA_d = inp("oneA", [128, ET, 128], dt.bfloat16)
    oneR_d = inp("oneR", [R, ET, 128], dt.bfloat16)

    out_d = nc.dram_tensor("out", [CHN, D], dt.float32, kind="ExternalOutput")

    xg_chunk = nc.dram_tensor("xg_chunk", [CHN, D2], dt.bfloat16)  # fp8 bytes
    xc_dram = nc.dram_tensor("xc_dram", [CHN, D], dt.float32)
    y_dram = nc.dram_tensor("y_dram", [CHN, D], dt.float32)
    y_dram_t = y_dram.rearrange("(t p) d -> p t d", p=128)
    xg_full = nc.dram_tensor("xg_full", [N, D2], dt.bfloat16,
                             addr_space="Shared")  # halves-major node layout

    x_own_t = x_own.rearrange("(t p) d -> p t d", p=128)
    xc_dram_t = xc_dram.rearrange("(t p) d -> p t d", p=128)

    with tile.TileContext(nc) as tc:
        nc.gpsimd.load_library(library_config.attnmlp)
        with tc.tile_pool(name="persist", bufs=1) as pp, \
             tc.tile_pool(name="scr", bufs=2) as sp, \
             tc.tile_pool(name="pm", bufs=3, space="PSUM") as pm, \
             tc.tile_pool(name="pt", bufs=2, space="PSUM") as pt, \
             tc.tile_pool(name="pagg", bufs=2, space="PSUM") as pagg, \
             tc.tile_pool(name="ptr", bufs=1, space="PSUM") as ptr:

            idxs = pp.tile([128, EPI // 16], dt.int16)
            nc.sync.dma_start(out=idxs[:], in_=idx_d[:])
            relT_s = pp.tile([128, KT2, 2, R], dt.float8e4)
            nc.sync.dma_start(out=relT_s[:], in_=relT_d[:])
            ident = pp.tile([128, 128], dt.bfloat16)
            nc.sync.dma_start(out=ident[:], in_=ident_d[:])
            wg1_s = pp.tile([128, D], dt.bfloat16)
            nc.sync.dma_start(out=wg1_s[:], in_=wg1_d[:])
            wg2_s = pp.tile([128, D], dt.bfloat16)
            nc.sync.dma_start(out=wg2_s[:], in_=wg2_d[:])
            bgr = pp.tile([128, 1], dt.float32)
            nc.sync.dma_start(out=bgr[:], in_=bgr_d[:])
            wp_s = pp.tile([128, KT, D], dt.bfloat16)
            nc.sync.dma_start(out=wp_s[:], in_=wp_d[:])
            bpr = pp.tile([128, D], dt.float32)
            nc.sync.dma_start(out=bpr[:], in_=bpr_d[:])
            st1 = pp.tile([128, NT], dt.float32)
            st2 = pp.tile([128, NT], dt.float32)

            for _rep in range(repeat):
                with tc.tile_pool(name="work", bufs=1) as wk:
                  for l in range(L):
                    XF = x_full if l == 0 else xg_full

                    blbr = wk.tile([128, HD], dt.bfloat16, tag="blbr")
                    nc.sync.dma_start(out=blbr[:], in_=blbr_d[l])

                    # rel_proj' = 64*(rel_emb @ We + bl + br) -> [64, 3072]
                    relp = wk.tile([R, HD], dt.bfloat16, tag="relp")
                    for ch in range(NCH):
                        sl = slice(ch * CW, (ch + 1) * CW)
                        wech = wk.tile([128, KT2, 2, CW], dt.float8e4,
                                       tag="sc", bufs=2)
                        nc.sync.dma_start(out=wech[:],
                                          in_=we_d[l][:, :, :, sl])
                        ps = pm.tile([128, CW], dt.float32, tag="pm")
                        for c in range(KT2):
                            nc.tensor.matmul(ps[:R, :], relT_s[:, c, :, :],
                                             wech[:, c, :, :],
                                             start=(c == 0),
                                             stop=(c == KT2 - 1),
                                             perf_mode=PM.DoubleRow)
                        rsc = sp.tile([R, CW], dt.bfloat16, tag="rsc", bufs=2)
                        nc.vector.tensor_scalar(out=rsc[:], in0=ps[:R, :],
                                                scalar1=1.0 / WSC,
                                                scalar2=None, op0=OP.mult)
                        nc.vector.tensor_tensor(out=relp[:, sl], in0=rsc[:],
                                                in1=blbr[:R, sl], op=OP.add)

                    wl_s = wk.tile([128, KT2, 2, HD], dt.float8e4, tag="wl_s")
                    nc.sync.dma_start(out=wl_s[:], in_=wl8_d[l])
                    wr_s = wk.tile([128, KT2, 2, HD], dt.float8e4, tag="wr_s")
                    nc.sync.dma_start(out=wr_s[:], in_=wr8_d[l])
                    attr = wk.tile([128, HD], dt.bfloat16, tag="attr")
                    nc.sync.dma_start(out=attr[:], in_=att_d[l])
                    blm = wk.tile([128, D], dt.float32, tag="blm")
                    nc.sync.dma_start(out=blm[:], in_=blm_d[l])
                    bor = wk.tile([128, D], dt.float32, tag="bor")
                    nc.sync.dma_start(out=bor[:], in_=bor_d[l])
                    xpre = wk.tile([128, NT, D], dt.float8e4, tag="xpre")
                    has_all = wk.tile([128, NT], dt.float32, tag="has_all")

                    for g in range(NT):
                        gs = slice(g * 128, (g + 1) * 128)
                        # one gather: transposed fp8 x rows for the group's
                        # padded edge sources + the group's own 128 nodes.
                        # 16-bit transpose granularity leaves k-pairs per
                        # partition: partition p holds k = 256c + 2p + j.
                        i0 = g * GI
                        gat = wk.tile([128, KT2, GI], dt.bfloat16, tag="gat",
                                      bufs=2)
                        nc.gpsimd.dma_gather(
                            gat[:], XF[:],
                            idxs[:, i0 // 16:(i0 + GI) // 16],
                            num_idxs=GI, num_idxs_reg=GI,
                            elem_size=D2, transpose=True)
                        # raw pair-interleaved fp8 bytes; each 128-idx
                        # window is a [128, 256] contiguous lhsT for the
                        # DoubleRowSwInterleave matmul mode
                        gat8 = gat[:].bitcast(dt.float8e4)

                        # xr_g = 64 * x_g @ Wr  (fp8 DoubleRow)
                        xr = wk.tile([128, HD], dt.bfloat16, tag="xr", bufs=2)
                        for ch in range(NCH):
                            sl = slice(ch * CW, (ch + 1) * CW)
                            ps = pm.tile([128, CW], dt.float32, tag="pm")
                            for c in range(KT2):
                                nc.tensor.matmul(
                                    ps[:], gat8[:, c, 2 * GE:2 * GI],
                                    wr_s[:, c, :, sl],
                                    start=(c == 0), stop=(c == KT2 - 1),
                                    perf_mode=PM.DoubleRowSwInterleave)
                            nc.any.tensor_copy(xr[:, sl], ps[:])

                        # per-group static one-hots
                        ts0 = g * TG
                        oneDg = wk.tile([128, TG, 128], dt.bfloat16, tag="oneDg",
                                        bufs=2)
                        nc.sync.dma_start(out=oneDg[:],
                                          in_=oneD_d[:, ts0:ts0 + TG, :])
                        oneAg = wk.tile([128, TG, 128], dt.bfloat16, tag="oneAg",
                                        bufs=2)
                        nc.sync.dma_start(out=oneAg[:],
                                          in_=oneA_d[:, ts0:ts0 + TG, :])
                        oneRg = wk.tile([R, TG, 128], dt.bfloat16, tag="oneRg",
                                        bufs=2)
                        nc.sync.dma_start(out=oneRg[:],
                                          in_=oneR_d[:, ts0:ts0 + TG, :])

                        xl_sb = wk.tile([128, TG, HD], dt.float8e4,
                                        tag="xl_sb", bufs=2)
                        exa = wk.tile([128, TG, H], dt.float32, tag="exa",
                                      bufs=2)
                        for kt in range(TG):
                            ew = slice(2 * kt * 128, 2 * (kt + 1) * 128)
                            # m chunks: 64*(xl + rel_proj[rel] + xr[dst]);
                            # Lrelu(scale=1/64) recovers true leaky-relu(m)
                            rt = wk.tile([128, HD], dt.bfloat16, tag="rt",
                                         bufs=2)
                            for ch in range(NCH):
                                sl = slice(ch * CW, (ch + 1) * CW)
                                mp = pm.tile([128, CW], dt.float32, tag="pm")
                                for c in range(KT2):
                                    nc.tensor.matmul(
                                        mp[:], gat8[:, c, ew],
                                        wl_s[:, c, :, sl],
                                        start=(c == 0), stop=False,
                                        perf_mode=PM.DoubleRowSwInterleave)
                                nc.any.tensor_copy(xl_sb[:, kt, sl], mp[:])
                                nc.tensor.matmul(mp[:], oneRg[:, kt, :],
                                                 relp[:, sl], start=False,
                                                 stop=False)
                                nc.tensor.matmul(mp[:], oneDg[:, kt, :],
                                                 xr[:, sl], start=False,
                                                 stop=True)
                                nc.scalar.activation(rt[:, sl], mp[:], AF.Lrelu,
                                                     scale=1.0 / WSC, alpha=0.2)
                            # logits = att . lrelu(m): one big multiply +
                            # one 3D reduce per edge tile
                            sc = wk.tile([128, HD], dt.float8e4, tag="sc",
                                         bufs=2)
                            nc.vector.tensor_tensor(out=sc[:], in0=attr[:],
                                                    in1=rt[:], op=OP.mult)
                            red = sp.tile([128, H], dt.float32, tag="red",
                                          bufs=4)
                            nc.vector.tensor_reduce(
                                out=red[:], in_=sc[:].rearrange(
                                    "p (h d) -> p h d", h=H),
                                axis=AX.X, op=OP.add)
                            nc.vector.tensor_copy(exa[:, kt, :], red[:])
                        # one Exp for all the group's edge tiles (avoids
                        # ACT table-set thrash against the Lrelu set)
                        ex_sb = wk.tile([128, TG, H], dt.bfloat16, tag="ex_sb",
                                        bufs=2)
                        exf = wk.tile([128, TG, H], dt.float32, tag="exf",
                                      bufs=2)
                        nc.scalar.activation(
                            exf[:].rearrange("p t h -> p (t h)"),
                            exa[:].rearrange("p t h -> p (t h)"), AF.Exp,
                            scale=1.0 / WSC)
                        nc.vector.tensor_copy(
                            ex_sb[:].rearrange("p t h -> p (t h)"),
                            exf[:].rearrange("p t h -> p (t h)"))

                        # segment softmax pieces (256 = 4*64 folds the head
                        # mean and the fp8 weight scale out of agg)
                        dn = pt.tile([128, H], dt.float32, tag="pt")
                        for kt in range(TG):
                            nc.tensor.matmul(dn[:], oneAg[:, kt, :],
                                             ex_sb[:, kt, :],
                                             start=(kt == 0), stop=(kt == TG - 1))
                        dn4 = sp.tile([128, H], dt.float32, tag="dn4")
                        nc.vector.tensor_scalar(out=dn4[:], in0=dn[:],
                                                scalar1=4.0 * WSC,
                                                scalar2=DEN_EPS,
                                                op0=OP.mult, op1=OP.add)
                        nc.vector.tensor_scalar(out=has_all[:, g:g + 1],
                                                in0=dn4[:, 0:1],
                                                scalar1=2.0 * DEN_EPS,
                                                scalar2=None, op0=OP.is_gt)
                        rden = sp.tile([128, H], dt.float32, tag="rden")
                        nc.vector.reciprocal(rden[:], dn4[:])
                        rden_bf = sp.tile([128, H], dt.bfloat16, tag="rden_bf")
                        nc.vector.tensor_copy(rden_bf[:], rden[:])

                        A_sb = wk.tile([128, TG, H, 128], dt.bfloat16,
                                       tag="A_sb", bufs=2)
                        for kt in range(TG):
                            re = pt.tile([128, H], dt.float32, tag="pt")
                            nc.tensor.matmul(re[:], oneDg[:, kt, :], rden_bf[:],
                                             start=True, stop=True)
                            re_f = sp.tile([128, H], dt.float32, tag="re_f")
                            nc.vector.tensor_copy(re_f[:], re[:])
                            for h in range(H):
                                nc.vector.tensor_scalar(
                                    out=A_sb[:, kt, h, :], in0=oneAg[:, kt, :],
                                    scalar1=exf[:, kt, h:h + 1],
                                    scalar2=re_f[:, h:h + 1],
                                    op0=OP.mult, op1=OP.mult)

                        # aggregate + head mean (0.25/64 folded via 256*den);
                        # gelu/residual deferred to the per-layer pass below
                        for j in range(2):
                            jsl = slice(j * 384, (j + 1) * 384)
                            ag = pagg.tile([128, 384], dt.float32, tag="pagg")
                            first = True
                            for kt in range(TG):
                                for h in range(H):
                                    nc.tensor.matmul(
                                        ag[:], A_sb[:, kt, h, :],
                                        xl_sb[:, kt, h * D + j * 384:
                                              h * D + (j + 1) * 384],
                                        start=first,
                                        stop=(kt == TG - 1 and h == H - 1))
                                    first = False
                            nc.vector.tensor_tensor(out=xpre[:, g, jsl],
                                                    in0=ag[:],
                                                    in1=bor[:, jsl], op=OP.add)

                    # ---- deferred per-layer pass: gelu + residual + stores;
                    # split in halves so the first half-AllGather overlaps
                    # the second half's group compute
                    XMST = x_own_t if l == 0 else xc_dram_t
                    for g in range(NT):
                        gs = slice(g * 128, (g + 1) * 128)
                        xcg = sp.tile([128, D], dt.float32, tag="xcg", bufs=2)
                        nc.sync.dma_start(out=xcg[:], in_=XMST[:, g, :])
                        u = sp.tile([128, D], dt.float32, tag="u", bufs=2)
                        nc.vector.tensor_scalar(out=u[:], in0=blm[:],
                                                scalar1=has_all[:, g:g + 1],
                                                scalar2=None, op0=OP.mult)
                        nc.vector.tensor_tensor(out=u[:], in0=xpre[:, g, :],
                                                in1=u[:], op=OP.add)
                        xn = sp.tile([128, D], dt.float32, tag="xn", bufs=2)
                        nc.scalar.activation(xn[:], u[:], AF.Gelu)
                        nc.vector.tensor_tensor(out=xn[:], in0=xn[:],
                                                in1=xcg[:], op=OP.add)
                        if l < L - 1:
                            nc.sync.dma_start(out=xc_dram_t[:, g, :], in_=xn[:])
                            xnb = sp.tile([128, D], dt.float8e4, tag="xnb",
                                          bufs=2)
                            nc.vector.tensor_copy(xnb[:], xn[:])
                            nc.sync.dma_start(
                                out=xg_chunk[gs, :],
                                in_=xnb[:].bitcast(dt.bfloat16))
                        else:
                            # fused gate + fuse + Wp projection + LN stats
                            # (xn = final x_graph, still on-chip)
                            xtt = sp.tile([128, D], dt.float32, tag="xtt",
                                          bufs=2)
                            nc.sync.dma_start(out=xtt[:], in_=x_own_t[:, g, :])
                            t1 = sp.tile([128, D], dt.float32, tag="t1")
                            nc.vector.tensor_tensor(out=t1[:], in0=xtt[:],
                                                    in1=wg1_s[:], op=OP.mult)
                            r1 = sp.tile([128, 1], dt.float32, tag="r1",
                                         bufs=2)
                            nc.vector.tensor_reduce(out=r1[:], in_=t1[:],
                                                    axis=AX.X, op=OP.add)
                            nc.vector.tensor_tensor(out=t1[:], in0=xn[:],
                                                    in1=wg2_s[:], op=OP.mult)
                            r2 = sp.tile([128, 1], dt.float32, tag="r2",
                                         bufs=2)
                            nc.vector.tensor_reduce(out=r2[:], in_=t1[:],
                                                    axis=AX.X, op=OP.add)
                            nc.vector.tensor_tensor(out=r2[:], in0=r2[:],
                                                    in1=r1[:], op=OP.add)
                            alph = sp.tile([128, 1], dt.float32, tag="alph",
                                           bufs=2)
                            nc.scalar.activation(alph[:], r2[:], AF.Sigmoid,
                                                 bias=bgr[:])
                            dif = sp.tile([128, D], dt.float32, tag="dif",
                                          bufs=2)
                            nc.vector.tensor_tensor(out=dif[:], in0=xn[:],
                                                    in1=xtt[:], op=OP.subtract)
                            nc.vector.tensor_scalar(out=dif[:], in0=dif[:],
                                                    scalar1=alph[:],
                                                    scalar2=None, op0=OP.mult)
                            nc.vector.tensor_tensor(out=dif[:], in0=dif[:],
                                                    in1=xtt[:], op=OP.add)
                            dif_bf = sp.tile([128, D], dt.bfloat16,
                                             tag="dif_bf")
                            nc.vector.tensor_copy(dif_bf[:], dif[:])
                            xfT = sp.tile([128, KT, 128], dt.bfloat16,
                                          tag="xfT")
                            for k in range(KT):
                                tp = ptr.tile([128, 128], dt.bfloat16,
                                              tag="ptr")
                                nc.tensor.transpose(
                                    tp[:], dif_bf[:, k * 128:(k + 1) * 128],
                                    ident[:])
                                nc.vector.tensor_copy(xfT[:, k, :], tp[:])
                            yg = sp.tile([128, D], dt.float32, tag="yg")
                            for j in range(2):
                                jsl = slice(j * 384, (j + 1) * 384)
                                yp = pm.tile([128, 384], dt.float32, tag="pm")
                                for k in range(KT):
                                    nc.tensor.matmul(yp[:], xfT[:, k, :],
                                                     wp_s[:, k, jsl],
                                                     start=(k == 0),
                                                     stop=(k == KT - 1))
                                nc.vector.tensor_tensor(out=yg[:, jsl],
                                                        in0=yp[:],
                                                        in1=bpr[:, jsl],
                                                        op=OP.add)
                            nc.sync.dma_start(out=y_dram_t[:, g, :], in_=yg[:])
                            scr1 = sp.tile([128, D], dt.float32, tag="scr1")
                            nc.scalar.activation(scr1[:], yg[:], AF.Identity,
                                                 accum_out=st1[:, g:g + 1])
                            nc.scalar.activation(scr1[:], yg[:], AF.Square,
                                                 accum_out=st2[:, g:g + 1])
                        if l < L - 1 and g == NT // 2 - 1 and "ag" not in abl:
                            nc.gpsimd.collective_compute(
                                "AllGather", OP.bypass,
                                ins=[xg_chunk[0:CHN // 2, :]],
                                outs=[xg_full[0:N // 2, :]],
                                replica_groups=[list(range(NC))])

                    if l < L - 1 and "ag" not in abl:
                        nc.gpsimd.collective_compute(
                            "AllGather", OP.bypass,
                            ins=[xg_chunk[CHN // 2:CHN, :]],
                            outs=[xg_full[N // 2:N, :]],
                            replica_groups=[list(range(NC))])

                # ------------- final tail: layernorm + gelu over the
                # y projections staged during the last layer's fused pass
                with tc.tile_pool(name="fin", bufs=1) as fp:
                    gmr = fp.tile([128, D], dt.float32, tag="gmr")
                    nc.sync.dma_start(out=gmr[:], in_=gmr_d[:])
                    btr = fp.tile([128, D], dt.float32, tag="btr")
                    nc.sync.dma_start(out=btr[:], in_=btr_d[:])

                    for nt in range(NT):
                        ns = slice(nt * 128, (nt + 1) * 128)
                        y = fp.tile([128, D], dt.float32, tag="y", bufs=2)
                        nc.sync.dma_start(out=y[:], in_=y_dram_t[:, nt, :])
                        mu = fp.tile([128, 1], dt.float32, tag="mu", bufs=2)
                        nc.vector.tensor_scalar(out=mu[:], in0=st1[:, nt:nt + 1],
                                                scalar1=1.0 / D, scalar2=None,
                                                op0=OP.mult)
                        msq = fp.tile([128, 1], dt.float32, tag="msq", bufs=2)
                        nc.vector.tensor_scalar(out=msq[:], in0=st2[:, nt:nt + 1],
                                                scalar1=1.0 / D, scalar2=None,
                                                op0=OP.mult)
                        var = fp.tile([128, 1], dt.float32, tag="var", bufs=2)
                        nc.vector.tensor_tensor(out=var[:], in0=mu[:], in1=mu[:],
                                                op=OP.mult)
                        nc.vector.tensor_tensor(out=var[:], in0=msq[:],
                                                in1=var[:], op=OP.subtract)
                        nc.vector.tensor_scalar(out=var[:], in0=var[:],
                                                scalar1=EPS_LN, scalar2=None,
                                                op0=OP.add)
                        sd = fp.tile([128, 1], dt.float32, tag="sd", bufs=2)
                        nc.scalar.activation(sd[:], var[:], AF.Sqrt)
                        rs = fp.tile([128, 1], dt.float32, tag="rs", bufs=2)
                        nc.vector.reciprocal(rs[:], sd[:])
                        mrs = fp.tile([128, 1], dt.float32, tag="mrs", bufs=2)
                        nc.vector.tensor_tensor(out=mrs[:], in0=mu[:], in1=rs[:],
                                                op=OP.mult)
                        yn = fp.tile([128, D], dt.float32, tag="yn", bufs=2)
                        nc.vector.tensor_scalar(out=yn[:], in0=y[:],
                                                scalar1=rs[:], scalar2=mrs[:],
                                                op0=OP.mult, op1=OP.subtract)
                        nc.vector.tensor_tensor(out=yn[:], in0=yn[:], in1=gmr[:],
                                                op=OP.mult)
                        nc.vector.tensor_tensor(out=yn[:], in0=yn[:], in1=btr[:],
                                                op=OP.add)
                        og = fp.tile([128, D], dt.float32, tag="og", bufs=2)
                        nc.scalar.activation(og[:], yn[:], AF.Gelu)
                        nc.sync.dma_start(out=out_d[ns, :], in_=og[:])


    nc.compile()
    return nc


_CACHE = {}


def kernel(**inputs):
    in_maps, TG = _preprocess(**inputs)
    if TG not in _CACHE:
        _CACHE[TG] = build_program(TG)
    nc = _CACHE[TG]
    res = run_bass_kernel_spmd(nc, in_maps, list(range(NC)))
    out = np.concatenate([res.results[c]["out"] for c in range(NC)], axis=0)
    return out


if __name__ == "__main__":
    pass

